# revision 1
# baseline (speedup 1.0000x reference)
"""Multi-head attention TRN2 Bass kernel (8 NeuronCores, SPMD).

Problem: B=4, S=1024, E=1024, H=16 heads of dim 64, fp32.
    Q = q @ Wq^T (per head), K, V likewise
    scores = Q K^T / 8 ; P = softmax(scores) ; ctx = P V
    out = concat_heads(ctx) @ Wo^T

Sharding: core c handles batch b = c // 2 and head-group g = c % 2
(8 heads each). Each core computes a partial output projection over its
512 concat features; the host sums the two partials per batch (the
"unshard" for a reduction sharding).

Device design (no on-device transpose anywhere, all matmuls at the full
1 cycle/row bf16 PE rate; the whole kernel is one software pipeline):
  - Host passes x^T [E, S], per-head-transposed weight blocks
    wqT/wkT/wvT [E, 512] and woT [512, E], all pre-cast to bf16
    (matmul operands only; every accumulation stays fp32 in PSUM).
  - Inputs are host-pre-tiled into their exact SBUF-resident layout so
    each tensor loads with one flat 2D DMA (contiguous 8-16KB per
    partition, minimal descriptors), issued in consumption order.
  - Q^T, K^T produced in [d, s] layout with head pairs stacked to
    M=128; V in natural [t, d] layout, augmented with a ones block so
    the P@V matmul also emits the softmax denominator, broadcast across
    64 partitions (even heads [V|ones], odd heads [ones|V] so the
    denominator lands on the partitions the normalizing multiply needs).
  - scores^T [t, s]: K=64 row-tiled matmul pairs run CONCURRENTLY in
    disjoint PE row groups (measured 3ns apart), writing the two banks
    of one [128, 1024] PSUM tile; one ACT exp per pair-tile amortizes
    the ~190ns ACTIVATE overhead. The attention phase is exp-bound on
    the scalar engine, so the V projection and output projections are
    scheduled to fill the tensor engine's slack under it.
  - PSUM (8 banks): pp_big 2x[128,1024] rotates Q/K projection groups,
    score tiles and output-projection groups; pp_v 2x[128,512] keeps the
    V projection independent; pp_ctx 2x[128,512] holds the ctx/denom
    accumulators.
  - softmax without max-subtraction (scores ~N(0,1): exp is safe);
    normalization = fast-approx reciprocal (custom DVE op, base
    partition 0 only) + one DVE multiply per head on the tiny ctx^T,
    with a cross-partition SBUF->SBUF DMA for the reciprocal broadcast.
"""

from contextlib import ExitStack

import ml_dtypes
import numpy as np

import concourse.bacc as bacc
import concourse.mybir as mybir
import concourse.tile as tile
from concourse.bass_utils import run_bass_kernel_spmd

B, S, E, H = 4, 1024, 1024, 16
HD = 64          # head dim
HPC = 8          # heads per core
NPAIR = 4        # head pairs per core
NET = 8          # e-tiles (E / 128)
NTT = 8          # t-tiles (S / 128)
P = 128

F32 = mybir.dt.float32
BF16 = mybir.dt.bfloat16
EXP = mybir.ActivationFunctionType.Exp
SCALE = 1.0 / 8.0  # 1/sqrt(HD)
BF = ml_dtypes.bfloat16


def _emit(nc, tc, ctx, aps):
    xqT, xkT, xvT, wqT, wkT, wvT, woT, out = aps

    xpool = ctx.enter_context(tc.tile_pool(name="xpool", bufs=3))
    wpool = ctx.enter_context(tc.tile_pool(name="wpool", bufs=3))
    const = ctx.enter_context(tc.tile_pool(name="const", bufs=1))
    etp = ctx.enter_context(tc.tile_pool(name="etp", bufs=16))
    obp = ctx.enter_context(tc.tile_pool(name="obp", bufs=3))
    rcp = ctx.enter_context(tc.tile_pool(name="rcp", bufs=8))
    pp_mm = ctx.enter_context(tc.tile_pool(name="pp_mm", bufs=2, space="PSUM"))
    pp_sc = ctx.enter_context(tc.tile_pool(name="pp_sc", bufs=2, space="PSUM"))
    pp_ctx = ctx.enter_context(tc.tile_pool(name="pp_ctx", bufs=2, space="PSUM"))

    wo_t = const.tile([P, 4096], BF16, name="wo_t")
    qt = const.tile([P, 4096], BF16, name="qt")
    kt = const.tile([P, 4096], BF16, name="kt")
    vaug = const.tile([P, 8192], BF16, name="vaug")
    cat = const.tile([P, 4096], BF16, name="cat")

    # ones blocks of the V augmentation (see module docstring)
    v4 = vaug[:, :].rearrange("p (j q c) -> p j q c", q=2, c=P)
    nc.gpsimd.memset(v4[:, :, 0, HD:P], 1.0)
    nc.gpsimd.memset(v4[:, :, 1, 0:HD], 1.0)

    def load_wx(wT, xT):
        w = wpool.tile([P, NET * 512], BF16, name="w", tag="wt")
        nc.sync.dma_start(out=w[:], in_=wT[:])
        x = xpool.tile([P, NET * 1024], BF16, name="x", tag="xt")
        half = NET * 512
        nc.sync.dma_start(out=x[:, 0:half], in_=xT[:, 0:half])
        nc.sync.dma_start(out=x[:, half:2 * half], in_=xT[:, half:2 * half])
        return w, x

    wq, xq = load_wx(wqT, xqT)
    wk, xk = load_wx(wkT, xkT)
    wv, xv = load_wx(wvT, xvT)
    nc.sync.dma_start(out=wo_t[:], in_=woT[:])

    # ---- Q/K projections: both s-halves interleave in one 2-bank tile,
    # so consecutive matmuls share each weight load and one [128,1024]
    # copy drains the pair. Q and K alternate per head pair so the
    # attention of pair 0 (and with it the critical exp stream on the
    # scalar engine) can start as soon as possible. ----
    def proj_pair(w, x, dst, p):
        for sh in range(2):
            ps = pp_mm.tile([P, 512], F32, name="ps", tag="mm")
            for et in range(NET):
                nc.tensor.matmul(
                    ps[:],
                    lhsT=w[:, et * 512 + p * P:et * 512 + (p + 1) * P],
                    rhs=x[:, et * 1024 + sh * 512:et * 1024 + (sh + 1) * 512],
                    start=(et == 0), stop=(et == NET - 1),
                )
            nc.vector.tensor_copy(
                dst[:, p * 1024 + sh * 512:p * 1024 + (sh + 1) * 512], ps[:])



    # ---- attention (emitted before the V projection: the exp stream on
    # the scalar engine is the phase's critical path and must start as
    # early as possible; V-projection matmuls fill PE slack under it and
    # the ctx matmuls wait on their vaug blocks via Tile deps) ----
    def normalize_a(ctx_ps, qcol):
        # ctx rows 0:64, denominator rows 64:128. reciprocal_approx_fast
        # only works at base partition 0: move the denominator down first.
        rA = rcp.tile([P, 512], F32, name="rA", tag="rc")
        rA2 = rcp.tile([P, 512], F32, name="rA2", tag="rc")
        nc.vector.tensor_copy(rA[HD:P, :], ctx_ps[HD:P, :])
        nc.sync.dma_start(out=rA[0:HD, :], in_=rA[HD:P, :])
        nc.vector.reciprocal_approx_fast(rA2[0:HD, :], rA[0:HD, :])
        nc.vector.tensor_mul(cat[0:HD, qcol:qcol + 512],
                             ctx_ps[0:HD, :], rA2[0:HD, :])

    def normalize_b(ctx_ps, qcol):
        # mirrored: denominator rows 0:64, ctx rows 64:128
        rB = rcp.tile([P, 512], F32, name="rB", tag="rc")
        nc.vector.reciprocal_approx_fast(rB[0:HD, :], ctx_ps[0:HD, :])
        nc.sync.dma_start(out=rB[HD:P, :], in_=rB[0:HD, :])
        nc.vector.tensor_mul(cat[HD:P, qcol:qcol + 512],
                             ctx_ps[HD:P, :], rB[HD:P, :])

    def attention_pair(sh, p):
            qcol = p * 1024 + sh * 512
            ctxA = pp_ctx.tile([P, 512], F32, name="ctxA", tag="ctx")
            ctxB = pp_ctx.tile([P, 512], F32, name="ctxB", tag="ctx")
            for tt in range(NTT):
                kcol = p * 1024 + tt * P
                sAB = pp_sc.tile([P, 1024], F32, name="sAB", tag="sc")
                nc.tensor.matmul(
                    sAB[:, 0:512],
                    lhsT=kt[0:HD, kcol:kcol + P],
                    rhs=qt[0:HD, qcol:qcol + 512],
                    start=True, stop=True)
                nc.tensor.matmul(
                    sAB[:, 512:1024],
                    lhsT=kt[HD:P, kcol:kcol + P],
                    rhs=qt[HD:P, qcol:qcol + 512],
                    start=True, stop=True)
                eAB = etp.tile([P, 1024], BF16, name="eAB", tag="et")
                nc.scalar.activation(eAB[:], sAB[:], EXP, scale=SCALE)
                bA = (tt * HPC + 2 * p) * P
                bB = bA + P
                nc.tensor.matmul(ctxA[:], lhsT=vaug[:, bA:bA + P],
                                 rhs=eAB[:, 0:512],
                                 start=(tt == 0), stop=(tt == NTT - 1))
                nc.tensor.matmul(ctxB[:], lhsT=vaug[:, bB:bB + P],
                                 rhs=eAB[:, 512:1024],
                                 start=(tt == 0), stop=(tt == NTT - 1))
            normalize_a(ctxA, qcol)
            normalize_b(ctxB, qcol)

    def outproj(sh):
        # partial over our 512 concat features. The first half runs on
        # the pp_mm rotation (it overlaps the still-running attention);
        # the last half runs on the score banks, which are free by then,
        # with both i-halves interleaved per 2-bank tile so the final
        # tail streams at full rate.
        if sh == 0:
            for j in range(4):
                st = sh * 4 + j
                for ih in range(2):
                    ps = pp_mm.tile([P, 512], F32, name="po", tag="mm")
                    for p4 in range(4):
                        nc.tensor.matmul(
                            ps[:],
                            lhsT=cat[:, p4 * 1024 + st * P:p4 * 1024 + (st + 1) * P],
                            rhs=wo_t[:, p4 * 1024 + ih * 512:p4 * 1024 + (ih + 1) * 512],
                            start=(p4 == 0), stop=(p4 == 3))
                    ob = obp.tile([P, 512], F32, name="ob", tag="ob")
                    nc.vector.tensor_copy(ob[:], ps[:])
                    nc.sync.dma_start(
                        out=out[st * P:(st + 1) * P, ih * 512:(ih + 1) * 512],
                        in_=ob[:])
        else:
            for j in range(4):
                st = sh * 4 + j
                ps = pp_sc.tile([P, 1024], F32, name="po2", tag="sc")
                # rotate the accumulation order so the in-flight groups
                # need the last head pair only for their final matmul
                for k4 in range(4):
                    p4 = (k4 + j) % 4 if j < 2 else k4
                    lhsT = cat[:, p4 * 1024 + st * P:p4 * 1024 + (st + 1) * P]
                    for ih in range(2):
                        nc.tensor.matmul(
                            ps[:, ih * 512:(ih + 1) * 512],
                            lhsT=lhsT,
                            rhs=wo_t[:, p4 * 1024 + ih * 512:p4 * 1024 + (ih + 1) * 512],
                            start=(k4 == 0), stop=(k4 == 3))
                ob = obp.tile([P, 1024], F32, name="ob2", tag="ob2")
                nc.vector.tensor_copy(ob[:], ps[:])
                nc.sync.dma_start(out=out[st * P:(st + 1) * P, :], in_=ob[:])

    # ---- V projection: natural [t, hd] layout into vaug blocks ----
    def vproj():
      for tt in range(NTT):
        ps = pp_mm.tile([P, 512], F32, name="psv", tag="mm")
        for et in range(NET):
            nc.tensor.matmul(
                ps[:],
                lhsT=xv[:, et * 1024 + tt * P:et * 1024 + (tt + 1) * P],
                rhs=wv[:, et * 512:(et + 1) * 512],
                start=(et == 0), stop=(et == NET - 1),
            )
        # psum cols h*64+d ; even heads -> block cols 0:64, odd -> 64:128
        dstt = vaug[:, tt * 1024:(tt + 1) * 1024].rearrange(
            "p (j q c) -> p j q c", q=2, c=P)
        srcv = ps[:].rearrange("p (j q c) -> p j q c", q=2, c=HD)
        nc.vector.tensor_copy(dstt[:, :, 0, 0:HD], srcv[:, :, 0, :])
        nc.vector.tensor_copy(dstt[:, :, 1, HD:P], srcv[:, :, 1, :])

    # Q/K projections interleave with the attention per head pair: pair
    # p's scores (both s-halves) depend only on pair p's projections, so
    # the exp stream starts right after pair 0 and stays fed while the
    # remaining projections and the V projection fill the PE. (Tile-pool
    # slots are granted in declaration order, which makes this emission
    # order the schedule.) The first output projection slots in before
    # the very last attention block to overlap its exp tail.
    proj_pair(wq, xq, qt, 0)
    proj_pair(wk, xk, kt, 0)
    vproj()
    for p in range(NPAIR):
        if p > 0:
            proj_pair(wq, xq, qt, p)
            proj_pair(wk, xk, kt, p)
        attention_pair(0, p)
        if p == NPAIR - 1:
            outproj(0)
        attention_pair(1, p)
    outproj(1)


_CACHE = {}


def build():
    if "nc" in _CACHE:
        return _CACHE["nc"]
    nc = bacc.Bacc("TRN2", target_bir_lowering=False, debug=False)
    xqT = nc.dram_tensor("xqT", [P, NET * S], BF16, kind="ExternalInput").ap()
    xkT = nc.dram_tensor("xkT", [P, NET * S], BF16, kind="ExternalInput").ap()
    xvT = nc.dram_tensor("xvT", [P, NET * S], BF16, kind="ExternalInput").ap()
    wqT = nc.dram_tensor("wqT", [P, NET * HPC * HD], BF16, kind="ExternalInput").ap()
    wkT = nc.dram_tensor("wkT", [P, NET * HPC * HD], BF16, kind="ExternalInput").ap()
    wvT = nc.dram_tensor("wvT", [P, NET * HPC * HD], BF16, kind="ExternalInput").ap()
    woT = nc.dram_tensor("woT", [P, 4 * E], BF16, kind="ExternalInput").ap()
    out = nc.dram_tensor("out", [S, E], F32, kind="ExternalOutput").ap()
    with tile.TileContext(nc) as tc, ExitStack() as ctx:
        _emit(nc, tc, ctx, (xqT, xkT, xvT, wqT, wkT, wvT, woT, out))
    nc.compile()
    _CACHE["nc"] = nc
    return nc


def make_in_maps(query, key, value, Wq, Wk, Wv, Wo):
    in_maps = []
    for c in range(8):
        b, g = divmod(c, 2)
        hs = slice(g * HPC, (g + 1) * HPC)

        def bf(a):
            return np.ascontiguousarray(a).astype(BF)

        def sbuf_tile(a):
            # [E_or_512, N] -> the SBUF-resident layout [128, n_et * N]:
            # row p, col et*N+c  =  a[et*128 + p, c]
            et = a.shape[0] // P
            return bf(a.reshape(et, P, -1).transpose(1, 0, 2).reshape(P, -1))

        # x^T [E, S]; w blocks [E, 512] with col h*64+d = W[g*8+h, d, e];
        # woT [512, E] with woT[hd, i] = Wo[i, g*512+hd]
        in_maps.append({
            "xqT": sbuf_tile(np.asarray(query[b], np.float32).T),
            "xkT": sbuf_tile(np.asarray(key[b], np.float32).T),
            "xvT": sbuf_tile(np.asarray(value[b], np.float32).T),
            "wqT": sbuf_tile(np.asarray(Wq[hs], np.float32).transpose(2, 0, 1).reshape(E, HPC * HD)),
            "wkT": sbuf_tile(np.asarray(Wk[hs], np.float32).transpose(2, 0, 1).reshape(E, HPC * HD)),
            "wvT": sbuf_tile(np.asarray(Wv[hs], np.float32).transpose(2, 0, 1).reshape(E, HPC * HD)),
            "woT": sbuf_tile(np.asarray(Wo[:, g * HPC * HD:(g + 1) * HPC * HD], np.float32).T),
        })
    return in_maps


def kernel(query, key, value, Wq, Wk, Wv, Wo):
    nc = build()
    in_maps = make_in_maps(query, key, value, Wq, Wk, Wv, Wo)
    res = run_bass_kernel_spmd(nc, in_maps, list(range(8))).results
    out = np.empty((B, S, E), np.float32)
    for b in range(B):
        out[b] = res[2 * b]["out"] + res[2 * b + 1]["out"]
    return out

